# revision 1
# baseline (speedup 1.0000x reference)
"""MultiHeadCrossAttentionFusion kernel for TRN2 (8 NeuronCores, data-parallel over batch).

Layout strategy per core (batch shard BS=1024):
  Phase A: x -> xT (PE transpose, bf16), QKV matmuls (bf16, natural layout) -> qkv DRAM
  Phase B: per 128-row tile: partition-expansion DMAs pack 8 samples x 16 heads onto
           partitions, attention done as packed [128,128] matmuls with block-diagonal
           softmax masking; output scattered back to natural layout; LN stats computed.
  Phase C: projection matmuls from centered-ca^T (PE-transposed), LN folded into
           host-precomputed weights; residual add; output.
"""
import sys
sys.path.insert(0, "/opt/trn_rl_repo")
import numpy as np
import ml_dtypes
from contextlib import ExitStack

import concourse.bass as bass
from concourse import bacc as _bacc
import concourse.mybir as mybir
from concourse.tile import TileContext
from concourse.bass_utils import run_bass_kernel_spmd

B, CD, HID, H, D = 8192, 2048, 1024, 16, 64
NCORES = 8
BS = B // NCORES          # 1024 rows per core
NB = BS // 128            # 8 b-tiles
KT = CD // 128            # 16 k-tiles for qkv matmul
NCH_Q = (3 * HID) // 512  # 6 n-chunks of qkv
CT = HID // 128           # 8 c-tiles for proj
NCH_P = CD // 512         # 4 n-chunks of proj
EPS = 1e-5
F32 = mybir.dt.float32
BF16 = mybir.dt.bfloat16
AL = mybir.AluOpType
AF = mybir.ActivationFunctionType


def _bc_ap(row_ap, p=128):
    return bass.AP(tensor=row_ap.tensor, offset=row_ap.offset,
                   ap=[[0, p]] + list(row_ap.ap)[1:])


def build_nc(with_bias=True, linearize=False):
    nc = _bacc.Bacc()
    dp = nc.declare_dram_parameter
    x_c = dp("x_c", [BS, CD], F32, isOutput=False)
    x_m = dp("x_m", [BS, CD], F32, isOutput=False)
    Wq_c = dp("Wq_c", [CD, 3 * HID], BF16, isOutput=False)
    Wq_m = dp("Wq_m", [CD, 3 * HID], BF16, isOutput=False)
    bq_c = dp("bq_c", [1, 3 * HID], F32, isOutput=False)
    bq_m = dp("bq_m", [1, 3 * HID], F32, isOutput=False)
    Wg_c = dp("Wg_c", [HID, CD], BF16, isOutput=False)   # g1-folded, permuted proj W
    Wg_m = dp("Wg_m", [HID, CD], BF16, isOutput=False)
    v_c = dp("v_c", [1, CD], F32, isOutput=False)        # be1@Wp + b_proj
    v_m = dp("v_m", [1, CD], F32, isOutput=False)
    un_c = dp("un_c", [1, CD], F32, isOutput=False)
    un_m = dp("un_m", [1, CD], F32, isOutput=False)
    mask8 = dp("mask8", [128, 128], F32, isOutput=False)  # block-diag 0 / -800
    identb = dp("identb", [128, 128], BF16, isOutput=False)
    ones_bf = dp("ones_bf", [128, 1], BF16, isOutput=False)
    onesr_bf = dp("onesr_bf", [1, 128], BF16, isOutput=False)
    onesr_f = dp("onesr_f", [1, 128], F32, isOutput=False)
    out_c = dp("out_c", [BS, CD], F32, isOutput=True)
    out_m = dp("out_m", [BS, CD], F32, isOutput=True)

    with TileContext(nc, linearize=linearize) as tc, ExitStack() as ctx:
        consts = ctx.enter_context(tc.tile_pool(name="consts", bufs=1))
        dram = ctx.enter_context(tc.tile_pool(name="dram", bufs=1, space="DRAM"))
        psT = ctx.enter_context(tc.tile_pool(name="psT", bufs=1, space="PSUM"))
        psQ = ctx.enter_context(tc.tile_pool(name="psQ", bufs=2, space="PSUM"))
        psS = ctx.enter_context(tc.tile_pool(name="psS", bufs=1, space="PSUM"))
        psCA = ctx.enter_context(tc.tile_pool(name="psCA", bufs=1, space="PSUM"))
        pA_cm = tc.tile_pool(name="pA", bufs=1)
        pA = pA_cm.__enter__()
        tmpA_cm = tc.tile_pool(name="tmpA", bufs=2)
        tmpA = tmpA_cm.__enter__()
        wst_cm = tc.tile_pool(name="wstp", bufs=2)
        wstp = wst_cm.__enter__()

        # ---- load constants into SBUF
        sb_mask = consts.tile([128, 128], F32)
        nc.sync.dma_start(sb_mask, mask8[:, :])
        sb_id = consts.tile([128, 128], BF16)
        nc.sync.dma_start(sb_id, identb[:, :])
        sb_ones = consts.tile([128, 1], BF16)
        nc.sync.dma_start(sb_ones, ones_bf[:, :])
        sb_o64 = consts.tile([64, 1], BF16)
        nc.sync.dma_start(sb_o64, ones_bf[0:64, :])
        sb_or_bf = consts.tile([1, 128], BF16)
        nc.sync.dma_start(sb_or_bf, onesr_bf[:, :])
        sb_or_f = consts.tile([1, 128], F32)
        nc.sync.dma_start(sb_or_f, onesr_f[:, :])

        # qkv natural-layout intermediates in DRAM (tracked by tile pool)
        qkvd = {
            "c": dram.tile([BS, 3 * HID], BF16, name="qkvd_c", tag="qkvd_c"),
            "m": dram.tile([BS, 3 * HID], BF16, name="qkvd_m", tag="qkvd_m"),
        }

        # ---- Phase A: xT build + QKV matmuls
        xT = {
            "c": pA.tile([128, KT, BS], BF16, name="xT_c", tag="xT_c"),
            "m": pA.tile([128, KT, BS], BF16, name="xT_m", tag="xT_m"),
        }
        sb_bq = {}
        for t, bq in (("c", bq_c), ("m", bq_m)) if with_bias else ():
            row = pA.tile([1, 3 * HID], F32, name=f"bqr_{t}", tag=f"bqr_{t}")
            nc.sync.dma_start(row, bq[:, :])
            rowb = pA.tile([1, 3 * HID], BF16, name=f"bqrb_{t}",
                           tag=f"bqrb_{t}")
            nc.vector.tensor_copy(out=rowb, in_=row)
            sb_bq[t] = pA.tile([128, 3 * HID], BF16, name=f"bqb_{t}",
                               tag=f"bqb_{t}")
            for ch in range(NCH_Q):
                bps = psQ.tile([128, 512], F32, tag="px", name="bps")
                nc.tensor.matmul(
                    bps, lhsT=sb_or_bf,
                    rhs=rowb[0:1, ch * 512:(ch + 1) * 512],
                    start=True, stop=True)
                nc.scalar.copy(
                    out=sb_bq[t][:, ch * 512:(ch + 1) * 512], in_=bps)
        for t, xin in (("c", x_c), ("m", x_m)):
            for bt in range(NB):
                xn = tmpA.tile([128, CD], F32, tag="xn")
                nc.sync.dma_start(xn, xin[bt * 128:(bt + 1) * 128, :])
                xb = tmpA.tile([128, CD], BF16, tag="xb")
                nc.vector.tensor_copy(out=xb, in_=xn)
                for kt in range(KT):
                    pt = psT.tile([128, 128], BF16, tag="pt")
                    nc.tensor.transpose(pt, xb[:, kt * 128:(kt + 1) * 128], sb_id)
                    nc.scalar.copy(
                        out=xT[t][:, kt, bt * 128:(bt + 1) * 128], in_=pt)

        for t, Wt in (("c", Wq_c), ("m", Wq_m)):
            for nch in range(NCH_Q):
                wst = wstp.tile([128, KT, 512], BF16, tag="wst")
                nc.sync.dma_start(
                    wst,
                    Wt[:, nch * 512:(nch + 1) * 512].rearrange(
                        "(kt p) n -> p kt n", p=128))
                for bt in range(NB):
                    px = psQ.tile([128, 512], F32, tag="px")
                    for kt in range(KT):
                        nc.tensor.matmul(
                            px, lhsT=xT[t][:, kt, bt * 128:(bt + 1) * 128],
                            rhs=wst[:, kt, :],
                            start=(kt == 0), stop=(kt == KT - 1))
                    qb = tmpA.tile([128, 512], BF16, tag="qb")
                    if with_bias:
                        nc.vector.tensor_tensor(
                            out=qb, in0=px,
                            in1=sb_bq[t][:, nch * 512:(nch + 1) * 512],
                            op=AL.add)
                    else:
                        nc.vector.tensor_copy(out=qb, in_=px)
                    nc.sync.dma_start(
                        qkvd[t][bt * 128:(bt + 1) * 128,
                                nch * 512:(nch + 1) * 512], qb)

        wst_cm.__exit__(None, None, None)
        tmpA_cm.__exit__(None, None, None)
        pA_cm.__exit__(None, None, None)
        keep = ctx.enter_context(tc.tile_pool(name="keep", bufs=1))
        apool = ctx.enter_context(tc.tile_pool(name="apool", bufs=2))
        spool = ctx.enter_context(tc.tile_pool(name="spool", bufs=3))
        stp = ctx.enter_context(tc.tile_pool(name="stp", bufs=4))

        # ---- Phase B: attention per b-tile per branch
        # r_all / caT_all persist to phase C
        r_all = keep.tile([128, 2 * NB], F32, tag="r_all")
        mu_all = keep.tile([1, 2 * NB * 128], F32, tag="mu_all")
        caT_all = keep.tile([128, 2 * NB * (H // 2), 128], BF16,
                            tag="caT_all")

        for bt in range(NB):
            for bri, (qs, ks) in enumerate((("c", "m"), ("m", "c"))):
                rows = slice(bt * 128, (bt + 1) * 128)
                qnat = apool.tile([128, HID], BF16, tag="qnat")
                nc.sync.dma_start(qnat, qkvd[qs][rows, 0:HID])
                knat = apool.tile([128, HID], BF16, tag="knat")
                nc.sync.dma_start(knat, qkvd[ks][rows, HID:2 * HID])
                vnat = apool.tile([128, HID], BF16, tag="vnat")
                nc.sync.dma_start(vnat, qkvd[ks][rows, 2 * HID:3 * HID])
                QTa = apool.tile([64, H, 128], BF16, tag="QTa")
                KTa = apool.tile([64, H, 128], BF16, tag="KTa")
                VTa = apool.tile([64, H, 128], BF16, tag="VTa")
                for nat, dstT in ((qnat, QTa), (knat, KTa), (vnat, VTa)):
                    for h in range(H):
                        pt = psT.tile([64, 128], BF16, tag="pt")
                        nc.tensor.transpose(
                            pt, nat[:, h * 64:(h + 1) * 64], sb_id)
                        nc.scalar.copy(out=dstT[:, h, :], in_=pt)
                # caT2: partitions (h%2)*64+d, free (h//2, b)
                caT2 = apool.tile([128, H // 2, 128], BF16, tag="caT2")
                for j in range(16):
                    bsl = slice(j * 8, (j + 1) * 8)
                    kpk = spool.tile([64, 128], BF16, tag="kpk")
                    nc.scalar.copy(
                        out=kpk, in_=KTa[:, :, bsl].rearrange("d g b -> d b g"))
                    qpk = spool.tile([64, 128], BF16, tag="qpk")
                    nc.scalar.copy(
                        out=qpk, in_=QTa[:, :, bsl].rearrange("d h b -> d b h"))
                    vpk = spool.tile([64, 128], BF16, tag="vpk")
                    nc.scalar.copy(
                        out=vpk, in_=VTa[:, :, bsl].rearrange("d g b -> d b g"))
                    sp = psS.tile([128, 128], F32, tag="sp")
                    nc.tensor.matmul(sp, lhsT=kpk, rhs=qpk,
                                     start=True, stop=True)
                    vp_ps = psT.tile([128, 64], BF16, tag="vp_ps")
                    nc.tensor.transpose(vp_ps, vpk, sb_id[0:64, 0:64])
                    vp = spool.tile([128, 64], BF16, tag="vp")
                    nc.scalar.copy(out=vp, in_=vp_ps)
                    sm = spool.tile([128, 128], F32, tag="sm")
                    nc.vector.tensor_tensor(
                        out=sm, in0=sp, in1=sb_mask, op=AL.add)
                    eT = spool.tile([128, 128], BF16, tag="eT")
                    nc.scalar.activation(eT, sm, AF.Exp, scale=0.125)
                    cu = psCA.tile([128, 65], F32, tag="cu")
                    nc.tensor.matmul(cu[:, 0:64], lhsT=eT, rhs=vp,
                                     start=True, stop=True)
                    nc.tensor.matmul(cu[:, 64:65], lhsT=eT, rhs=sb_ones,
                                     start=True, stop=True)
                    rcz = stp.tile([128, 1], F32, tag="rcz")
                    nc.vector.reciprocal(rcz, cu[:, 64:65])
                    caj = spool.tile([128, 64], BF16, tag="caj")
                    nc.vector.tensor_scalar(
                        out=caj, in0=cu[:, 0:64], scalar1=rcz,
                        scalar2=None, op0=AL.mult)
                    ct_ps = psT.tile([64, 128], BF16, tag="pt")
                    nc.tensor.transpose(ct_ps, caj, sb_id)
                    # psum cols are (bi, h); h -> (hp, parity)
                    for par in range(2):
                        srcv = ct_ps.rearrange(
                            "d (b hp two) -> d b hp two", b=8, two=2)[
                            :, :, :, par]
                        dstv = caT2[par * 64:(par + 1) * 64, :, bsl]\
                            .rearrange("d hp b -> d b hp")
                        nc.scalar.copy(out=dstv, in_=srcv)
                # stats rows via ones-matmuls over caT2
                sq = apool.tile([128, H // 2, 128], BF16, tag="sqq")
                nc.vector.tensor_tensor(out=sq, in0=caT2, in1=caT2,
                                        op=AL.mult)
                mrow = psCA.tile([1, 128], F32, tag="mrow")
                srow = psCA.tile([1, 128], F32, tag="srow")
                for hp in range(H // 2):
                    nc.tensor.matmul(mrow, lhsT=sb_ones, rhs=caT2[:, hp, :],
                                     start=(hp == 0), stop=(hp == 7))
                    nc.tensor.matmul(srow, lhsT=sb_ones, rhs=sq[:, hp, :],
                                     start=(hp == 0), stop=(hp == 7))
                idx = bt * 2 + bri
                murow = stp.tile([1, 128], F32, tag="murow")
                nc.vector.tensor_scalar(
                    out=murow, in0=mrow, scalar1=1.0 / HID, scalar2=None,
                    op0=AL.mult)
                mu2 = stp.tile([1, 128], F32, tag="mu2")
                nc.vector.tensor_tensor(out=mu2, in0=murow, in1=murow,
                                        op=AL.mult)
                vvr = stp.tile([1, 128], F32, tag="vvr")
                nc.vector.tensor_scalar(
                    out=vvr, in0=srow, scalar1=1.0 / HID, scalar2=EPS,
                    op0=AL.mult, op1=AL.add)
                vv2 = stp.tile([1, 128], F32, tag="vv2")
                nc.vector.tensor_tensor(out=vv2, in0=vvr, in1=mu2,
                                        op=AL.subtract)
                sd = stp.tile([1, 128], F32, tag="sd")
                nc.scalar.activation(sd, vv2, AF.Sqrt)
                rrow = stp.tile([1, 128], F32, tag="rrow")
                nc.vector.reciprocal(rrow, sd)
                rrow_bf = stp.tile([1, 128], BF16, tag="rrow_bf")
                nc.vector.tensor_copy(out=rrow_bf, in_=rrow)
                rc_ps = psT.tile([128, 1], BF16, tag="vp_ps")
                nc.tensor.transpose(rc_ps, rrow_bf, sb_id[0:1, 0:1])
                nc.scalar.copy(out=r_all[:, idx:idx + 1], in_=rc_ps)
                nc.vector.tensor_copy(
                    out=mu_all[:, idx * 128:(idx + 1) * 128], in_=murow)
                nc.vector.tensor_copy(
                    out=caT_all[:, idx * (H // 2):(idx + 1) * (H // 2), :],
                    in_=caT2)

        # ---- Phase C: projection + residual
        wgp = ctx.enter_context(tc.tile_pool(name="wgp", bufs=1))
        tmpC = ctx.enter_context(tc.tile_pool(name="tmpC", bufs=3))
        ung = {}
        for t, un in (("c", un_c), ("m", un_m)):
            ung[t] = wgp.tile([1, CD], F32, name=f"ung_{t}", tag=f"ung_{t}")
            nc.sync.dma_start(ung[t], un[:, :])
        sb_v = {}
        for t, vv in (("c", v_c), ("m", v_m)) if with_bias else ():
            row = wgp.tile([1, CD], F32, name=f"vr_{t}", tag=f"vr_{t}")
            nc.sync.dma_start(row, vv[:, :])
            sb_v[t] = wgp.tile([128, CD], F32, name=f"vb_{t}", tag=f"vb_{t}")
            for ch in range(NCH_P):
                vps = psQ.tile([128, 512], F32, tag="px", name="vps")
                nc.tensor.matmul(
                    vps, lhsT=sb_or_f,
                    rhs=row[0:1, ch * 512:(ch + 1) * 512],
                    start=True, stop=True)
                nc.scalar.copy(
                    out=sb_v[t][:, ch * 512:(ch + 1) * 512], in_=vps)
        for nch in range(NCH_P):
            wg = {}
            for t, Wgt in (("c", Wg_c), ("m", Wg_m)):
                w = wgp.tile([128, CT, 512], BF16, name=f"wg_{t}", tag=f"wg_{t}")
                nc.sync.dma_start(
                    w, Wgt[:, nch * 512:(nch + 1) * 512].rearrange(
                        "(ct p) n -> p ct n", p=128))
                wg[t] = w
            for bt in range(NB):
                for bi, t in enumerate(("c", "m")):
                    idx = bt * 2 + bi
                    xres = tmpC.tile([128, 512], F32, tag="xres")
                    xin = x_c if t == "c" else x_m
                    nc.sync.dma_start(
                        xres, xin[bt * 128:(bt + 1) * 128,
                                  nch * 512:(nch + 1) * 512])
                    px = psQ.tile([128, 512], F32, tag="px")
                    for ct in range(CT):
                        nc.tensor.matmul(
                            px, lhsT=caT_all[:, idx * CT + ct, :],
                            rhs=wg[t][:, ct, :],
                            start=(ct == 0), stop=False)
                    nc.tensor.matmul(
                        px, lhsT=mu_all[:, idx * 128:(idx + 1) * 128],
                        rhs=ung[t][:, nch * 512:(nch + 1) * 512],
                        start=False, stop=True)
                    t1 = tmpC.tile([128, 512], F32, tag="t1")
                    nc.scalar.activation(
                        t1, px, AF.Copy, scale=r_all[:, idx:idx + 1])
                    t2 = tmpC.tile([128, 512], F32, tag="t2")
                    nc.vector.tensor_tensor(out=t2, in0=t1, in1=xres, op=AL.add)
                    if with_bias:
                        ot = tmpC.tile([128, 512], F32, tag="ot")
                        nc.vector.tensor_tensor(
                            out=ot, in0=t2,
                            in1=sb_v[t][:, nch * 512:(nch + 1) * 512],
                            op=AL.add)
                    else:
                        ot = t2
                    outt = out_c if t == "c" else out_m
                    nc.sync.dma_start(
                        outt[bt * 128:(bt + 1) * 128,
                             nch * 512:(nch + 1) * 512], ot)
    return nc


_NC = {}


def _get_nc(with_bias):
    if with_bias not in _NC:
        nc = build_nc(with_bias=with_bias)
        if not nc.is_finalized():
            nc.finalize()
        _NC[with_bias] = nc
    return _NC[with_bias]


def _host_prep(inputs):
    f32 = np.float32
    bf = ml_dtypes.bfloat16
    g = {k: np.asarray(v) for k, v in inputs.items()}
    # permutation: device ca column c_dev = h*64+d  <->  ref column c_ref = d*16+h
    cdev = np.arange(HID)
    hp_t, p_t = cdev // 128, cdev % 128
    h_t = 2 * hp_t + (p_t // 64)
    d_t = p_t % 64
    pr = d_t * H + h_t                   # ref col for each (ct,partition) row
    consts = {}
    for t, (Wp, bp, g1, be1) in (
            ("c", ("W_cproj", "b_cproj", "g1", "be1")),
            ("m", ("W_mproj", "b_mproj", "g2", "be2"))):
        W = np.asarray(g[Wp], f32)[pr, :]          # [HID, CD] permuted
        g1d = np.asarray(g[g1], f32)[pr]
        be1d = np.asarray(g[be1], f32)[pr]
        consts[f"Wg_{t}"] = np.ascontiguousarray(
            (g1d[:, None] * W)).astype(bf)
        consts[f"v_{t}"] = (be1d @ W + np.asarray(g[bp], f32)).reshape(1, CD)\
            .astype(f32)
        consts[f"un_{t}"] = (-(g1d[:, None] * W).sum(0)).reshape(1, CD)\
            .astype(f32)
    consts["Wq_c"] = np.asarray(g["W_cqkv"], f32).astype(bf)
    consts["Wq_m"] = np.asarray(g["W_mqkv"], f32).astype(bf)
    consts["bq_c"] = np.asarray(g["b_cqkv"], f32).reshape(1, 3 * HID)
    consts["bq_m"] = np.asarray(g["b_mqkv"], f32).reshape(1, 3 * HID)
    p = np.arange(128)
    consts["mask8"] = np.where(
        (p[:, None] // H) == (p[None, :] // H), 0.0, -800.0).astype(f32)
    consts["identb"] = np.eye(128).astype(bf)
    consts["ones_bf"] = np.ones((128, 1)).astype(bf)
    consts["onesr_bf"] = np.ones((1, 128)).astype(bf)
    consts["onesr_f"] = np.ones((1, 128)).astype(f32)
    return g, consts


def kernel(**inputs):
    g, consts = _host_prep(inputs)
    xc = np.ascontiguousarray(np.asarray(g["cnn_out"], np.float32))
    xm = np.ascontiguousarray(np.asarray(g["mlp_out"], np.float32))
    wb = (np.abs(consts["bq_c"]).max() > 0 or np.abs(consts["bq_m"]).max() > 0
          or np.abs(consts["v_c"]).max() > 0 or np.abs(consts["v_m"]).max() > 0)
    nc = _get_nc(bool(wb))
    in_maps = []
    for i in range(NCORES):
        m = dict(consts)
        m["x_c"] = xc[i * BS:(i + 1) * BS]
        m["x_m"] = xm[i * BS:(i + 1) * BS]
        in_maps.append(m)
    res = run_bass_kernel_spmd(nc, in_maps, list(range(NCORES))).results
    out_c = np.concatenate([np.asarray(res[i]["out_c"]) for i in range(NCORES)], 0)
    out_m = np.concatenate([np.asarray(res[i]["out_m"]) for i in range(NCORES)], 0)
    return (out_c.astype(np.float32), out_m.astype(np.float32))



# revision 13
# speedup vs baseline: 1.9488x; 1.9488x over previous
"""MultiHeadCrossAttentionFusion kernel for TRN2 (8 NeuronCores, data-parallel).

v2 design (per core, batch shard BS=1024):
  Phase A: xT via DMA-transpose (bf16 x from host). Q,K computed d-major
           directly on the PE (lhsT=W chunk, rhs=xT) -> QP/KP [64d, H, B] in
           SBUF (branch halves stacked on partitions). V computed in natural
           layout and written to DRAM.
  Phase B: per 128-row tile x branch: VP [(b8,g), j, d|1] gathered by DMA from
           v DRAM with a ones column appended. Per j-group of 8 samples:
           score matmul fed by strided APs (no packing copies), exp on ACT,
           multiplicative block-diag mask, ca matmul with rhs=VP[:,j,0:65]
           giving ca rows (b8,h) + softmax denominator in one matmul.
           Per-partition softmax scale, PE transpose, parity-split into caT2.
           LN stats via ones-matmuls; rsqrt as exp(-0.5*ln(v)) to stay in one
           ACT table set; 1/sigma kept as a per-sample column r_all.
  Phase C: proj matmuls (lhsT=caT2), mu*un correction, r_col scale on ACT,
           residual add from bf16 x, bf16 outputs (host casts back to f32).
"""
import sys
sys.path.insert(0, "/opt/trn_rl_repo")
import numpy as np
import ml_dtypes
from contextlib import ExitStack

import concourse.bass as bass
from concourse import bacc as _bacc
import concourse.mybir as mybir
from concourse.tile import TileContext
from concourse.bass_utils import run_bass_kernel_spmd

B, CD, HID, H, D = 8192, 2048, 1024, 16, 64
NCORES = 8
BS = B // NCORES          # 1024 rows per core
KT = CD // 128            # 16 k-tiles
EPS = 1e-5
F32 = mybir.dt.float32
BF16 = mybir.dt.bfloat16
AL = mybir.AluOpType
AF = mybir.ActivationFunctionType


def _bc(ap, p):
    """Broadcast a [1, ...] AP across p partitions."""
    return bass.AP(tensor=ap.tensor, offset=ap.offset,
                   ap=[[0, p]] + list(ap.ap)[1:])


def build_nc(with_bias=False, bs=BS, dbg=False):
    nb = bs // 128
    bchunk = min(512, bs)
    nbc = bs // bchunk
    nc = _bacc.Bacc()
    dp = nc.declare_dram_parameter
    if dbg:
        dbg_T1 = dp("dbg_T1", [128, H * bs], BF16, isOutput=True)
        dbg_T2 = dp("dbg_T2", [128, H * bs], BF16, isOutput=True)
        dbg_caT2 = dp("dbg_caT2", [128, 2 * nb * (H // 2) * 128], BF16,
                      isOutput=True)
        dbg_r = dp("dbg_r", [128, 2 * nb], F32, isOutput=True)
        dbg_mu = dp("dbg_mu", [1, 2 * nb * 128], BF16, isOutput=True)
        dbg_v = dp("dbg_v", [bs, HID], BF16, isOutput=True)
    x_c = dp("x_c", [bs, CD], BF16, isOutput=False)
    x_m = dp("x_m", [bs, CD], BF16, isOutput=False)
    Wq_c = dp("Wq_c", [CD, 3 * HID], BF16, isOutput=False)
    Wq_m = dp("Wq_m", [CD, 3 * HID], BF16, isOutput=False)
    Wg_c = dp("Wg_c", [HID, CD], BF16, isOutput=False)
    Wg_m = dp("Wg_m", [HID, CD], BF16, isOutput=False)
    un_c = dp("un_c", [1, CD], BF16, isOutput=False)
    un_m = dp("un_m", [1, CD], BF16, isOutput=False)
    mask01 = dp("mask01", [128, 128], BF16, isOutput=False)
    ones16 = dp("ones16", [128, 16], BF16, isOutput=False)
    ones_col = dp("ones_col", [128, 1], BF16, isOutput=False)
    identb = dp("identb", [128, 128], BF16, isOutput=False)
    ones_f = dp("ones_f", [1, 1], F32, isOutput=False)
    out_c = dp("out_c", [bs, CD], BF16, isOutput=True)
    out_m = dp("out_m", [bs, CD], BF16, isOutput=True)
    if with_bias:
        bq_c = dp("bq_c", [1, 3 * HID], F32, isOutput=False)
        bq_m = dp("bq_m", [1, 3 * HID], F32, isOutput=False)
        vb_c = dp("vb_c", [1, CD], F32, isOutput=False)
        vb_m = dp("vb_m", [1, CD], F32, isOutput=False)

    xin_d = {"c": x_c, "m": x_m}
    Wq_d = {"c": Wq_c, "m": Wq_m}
    Wg_d = {"c": Wg_c, "m": Wg_m}
    un_d = {"c": un_c, "m": un_m}
    out_d = {"c": out_c, "m": out_m}
    # partition bases: T1 holds q_c @0, q_m @64; T2 holds k_m @0, k_c @64
    qbase = {"c": 0, "m": 64}
    kbase = {"m": 0, "c": 64}

    with TileContext(nc) as tc, ExitStack() as ctx:
        consts = ctx.enter_context(tc.tile_pool(name="consts", bufs=1))
        dram = ctx.enter_context(tc.tile_pool(name="dram", bufs=1, space="DRAM"))
        keep = ctx.enter_context(tc.tile_pool(name="keep", bufs=1))

        sb_mask = consts.tile([128, 128], BF16)
        nc.sync.dma_start(sb_mask, mask01[:, :])
        sb_id = consts.tile([128, 128], BF16)
        nc.sync.dma_start(sb_id, identb[:, :])
        sb_o16 = consts.tile([128, 16], BF16)
        nc.sync.dma_start(sb_o16, ones16[:, :])
        sb_ones = consts.tile([128, 1], BF16)
        nc.sync.dma_start(sb_ones, ones_col[:, :])
        sb_of = consts.tile([1, 1], F32)
        nc.sync.dma_start(sb_of, ones_f[:, :])
        sb_un = {}
        for t in ("c", "m"):
            sb_un[t] = consts.tile([1, CD], BF16, name=f"un_{t}", tag=f"un_{t}")
            nc.sync.dma_start(sb_un[t], un_d[t][:, :])
        sb_bqc = {}
        sb_bqr = {}
        sb_vb = {}
        if with_bias:
            for t, bq, vb in (("c", bq_c, vb_c), ("m", bq_m, vb_m)):
                sb_bqc[t] = consts.tile([128, 2 * HID // 128], F32,
                                        name=f"bqc_{t}", tag=f"bqc_{t}")
                nc.sync.dma_start(
                    sb_bqc[t],
                    bq[:, 0:2 * HID].rearrange("o (c p) -> (o p) c", p=128))
                sb_bqr[t] = consts.tile([1, 3 * HID], F32,
                                        name=f"bqr_{t}", tag=f"bqr_{t}")
                nc.sync.dma_start(sb_bqr[t], bq[:, :])
                sb_vb[t] = consts.tile([1, CD], F32,
                                       name=f"vb_{t}", tag=f"vb_{t}")
                nc.sync.dma_start(sb_vb[t], vb[:, :])

        v_dram = {
            t: dram.tile([bs, HID], BF16, name=f"vd_{t}", tag=f"vd_{t}")
            for t in ("c", "m")
        }

        # packed layout [part, jblock, b8, head]: per-j matmul operands are
        # contiguous 128-column slices (walrus: one free dim only)
        T1 = keep.tile([128, bs // 8, 8, H], BF16, tag="T1")   # QP
        T2 = keep.tile([128, bs // 8, 8, H], BF16, tag="T2")   # KP
        caT2 = keep.tile([128, 2 * nb * (H // 2), 128], BF16, tag="caT2")
        murow_bf = keep.tile([1, 2 * nb * 128], BF16, tag="murow_bf")
        r_all = keep.tile([128, 2 * nb], F32, tag="r_all")

        # ---------------- Phase A ----------------
        paX_cm = tc.tile_pool(name="paX", bufs=1)
        paX = paX_cm.__enter__()
        paW_cm = tc.tile_pool(name="paW", bufs=2)
        paW = paW_cm.__enter__()
        paT_cm = tc.tile_pool(name="paT", bufs=2)
        paT = paT_cm.__enter__()
        psA_cm = tc.tile_pool(name="psA", bufs=2, space="PSUM")
        psA = psA_cm.__enter__()

        for t in ("c", "m"):
            xT = paX.tile([128, KT, bs], BF16, tag="xT")
            for kt in range(KT):
                nc.sync.dma_start_transpose(
                    xT[:, kt, :], xin_d[t][:, kt * 128:(kt + 1) * 128])
            # q, k in d-major layout
            for sect, dtile, dbase in (("q", T1, qbase[t]),
                                       ("k", T2, kbase[t])):
                off = 0 if sect == "q" else HID
                for ch in range(2):
                    wst = paW.tile([128, KT, 512], BF16, tag="wst")
                    nc.sync.dma_start(
                        wst,
                        Wq_d[t][:, off + ch * 512:off + (ch + 1) * 512]
                        .rearrange("(kt p) n -> p kt n", p=128))
                    for hs in range(4):
                        hp = ch * 4 + hs
                        cc = off // 128 + ch * 4 + hs
                        for bc in range(nbc):
                            px = psA.tile([128, bchunk], F32, tag="pxA")
                            bsl = slice(bc * bchunk, (bc + 1) * bchunk)
                            for kt in range(KT):
                                nc.tensor.matmul(
                                    px,
                                    lhsT=wst[:, kt, hs * 128:(hs + 1) * 128],
                                    rhs=xT[:, kt, bsl],
                                    start=(kt == 0), stop=(kt == KT - 1))
                            jbc = bchunk // 8
                            for par in range(2):
                                src = px[par * 64:(par + 1) * 64, :]\
                                    .rearrange("p (jb b8) -> p jb b8", b8=8)
                                dst = dtile[dbase:dbase + 64,
                                            bc * jbc:(bc + 1) * jbc,
                                            :, 2 * hp + par]
                                if with_bias:
                                    nc.vector.tensor_scalar(
                                        out=dst, in0=src,
                                        scalar1=sb_bqc[t][
                                            par * 64:(par + 1) * 64,
                                            cc:cc + 1],
                                        scalar2=None, op0=AL.add)
                                else:
                                    nc.vector.tensor_copy(out=dst, in_=src)
            # v in natural layout -> DRAM
            for ch in range(2):
                wst = paW.tile([128, KT, 512], BF16, tag="wst")
                nc.sync.dma_start(
                    wst,
                    Wq_d[t][:, 2 * HID + ch * 512:2 * HID + (ch + 1) * 512]
                    .rearrange("(kt p) n -> p kt n", p=128))
                for bt in range(nb):
                    px = psA.tile([128, 512], F32, tag="pxV")
                    for kt in range(KT):
                        nc.tensor.matmul(
                            px,
                            lhsT=xT[:, kt, bt * 128:(bt + 1) * 128],
                            rhs=wst[:, kt, :],
                            start=(kt == 0), stop=(kt == KT - 1))
                    vs = paT.tile([128, 512], BF16, tag="vstage")
                    if with_bias:
                        nc.vector.tensor_tensor(
                            out=vs, in0=px,
                            in1=_bc(sb_bqr[t][:, 2 * HID + ch * 512:
                                              2 * HID + (ch + 1) * 512], 128),
                            op=AL.add)
                    else:
                        nc.vector.tensor_copy(out=vs, in_=px)
                    nc.sync.dma_start(
                        v_dram[t][bt * 128:(bt + 1) * 128,
                                  ch * 512:(ch + 1) * 512], vs)

        psA_cm.__exit__(None, None, None)
        paT_cm.__exit__(None, None, None)
        paW_cm.__exit__(None, None, None)
        paX_cm.__exit__(None, None, None)

        # ---------------- Phase B ----------------
        pb = ctx.enter_context(tc.tile_pool(name="pb", bufs=2))
        spool = ctx.enter_context(tc.tile_pool(name="spool", bufs=3))
        stp = ctx.enter_context(tc.tile_pool(name="stp", bufs=4))
        psS_cm = tc.tile_pool(name="psS", bufs=2, space="PSUM")
        psS = psS_cm.__enter__()
        psC_cm = tc.tile_pool(name="psC", bufs=2, space="PSUM")
        psC = psC_cm.__enter__()
        psT_cm = tc.tile_pool(name="psT", bufs=1, space="PSUM")
        psT = psT_cm.__enter__()

        for bt in range(nb):
            for bri, (qs, ks) in enumerate((("c", "m"), ("m", "c"))):
                idx = bt * 2 + bri
                qb = qbase[qs]
                kb = kbase[ks]
                vp = pb.tile([128, 16, 65], BF16, tag="vp")
                nc.sync.dma_start(
                    vp[:, :, 0:64],
                    v_dram[ks][bt * 128:(bt + 1) * 128, :].rearrange(
                        "(j b8) (g d) -> (b8 g) j d", b8=8, g=16))
                nc.vector.tensor_copy(out=vp[:, :, 64:65], in_=sb_o16)

                for j in range(16):
                    jb = bt * 16 + j
                    sp = psS.tile([128, 128], F32, tag="sp")
                    nc.tensor.matmul(
                        sp,
                        lhsT=T2[kb:kb + 64, jb, :, :],
                        rhs=T1[qb:qb + 64, jb, :, :],
                        start=True, stop=True)
                    eT = spool.tile([128, 128], BF16, tag="eT")
                    nc.scalar.activation(eT, sp, AF.Exp, scale=0.125)
                    eTm = spool.tile([128, 128], BF16, tag="eTm")
                    nc.vector.tensor_tensor(out=eTm, in0=eT, in1=sb_mask,
                                            op=AL.mult)
                    cu = psC.tile([128, 65], F32, tag="cu")
                    nc.tensor.matmul(cu, lhsT=eTm, rhs=vp[:, j, :],
                                     start=True, stop=True)
                    rcz = stp.tile([128, 1], F32, tag="rcz")
                    nc.vector.reciprocal(rcz, cu[:, 64:65])
                    caj = spool.tile([128, 64], BF16, tag="caj")
                    nc.vector.tensor_scalar(out=caj, in0=cu[:, 0:64],
                                            scalar1=rcz, scalar2=None,
                                            op0=AL.mult)
                    ct = psC.tile([64, 128], BF16, tag="ct")
                    nc.tensor.transpose(ct, caj, sb_id)
                    for par in range(2):
                        src = ct.rearrange(
                            "d (b8 hp two) -> d hp b8 two",
                            b8=8, two=2)[:, :, :, par]
                        dst = caT2[par * 64:(par + 1) * 64,
                                   idx * 8:(idx + 1) * 8,
                                   j * 8:(j + 1) * 8]
                        nc.vector.tensor_copy(out=dst, in_=src)

                # LN stats for this tile-branch (sequential groups, one bank)
                cslice = caT2[:, idx * 8:(idx + 1) * 8, :]
                sq = spool.tile([128, H // 2, 128], BF16, tag="sq")
                nc.vector.tensor_tensor(out=sq, in0=cslice, in1=cslice,
                                        op=AL.mult)
                stat = psT.tile([1, 256], F32, tag="stat")
                mrow = stat[:, 0:128]
                srow = stat[:, 128:256]
                for hp in range(H // 2):
                    nc.tensor.matmul(mrow, lhsT=sb_ones,
                                     rhs=caT2[:, idx * 8 + hp, :],
                                     start=(hp == 0), stop=(hp == 7))
                for hp in range(H // 2):
                    nc.tensor.matmul(srow, lhsT=sb_ones, rhs=sq[:, hp, :],
                                     start=(hp == 0), stop=(hp == 7))
                murow = stp.tile([1, 128], F32, tag="murow")
                nc.vector.tensor_scalar(out=murow, in0=mrow,
                                        scalar1=1.0 / HID, scalar2=None,
                                        op0=AL.mult)
                t2r = stp.tile([1, 128], F32, tag="t2r")
                nc.vector.tensor_scalar(out=t2r, in0=srow, scalar1=1.0 / HID,
                                        scalar2=EPS, op0=AL.mult, op1=AL.add)
                mu2 = stp.tile([1, 128], F32, tag="mu2")
                nc.vector.tensor_tensor(out=mu2, in0=murow, in1=murow,
                                        op=AL.mult)
                vv = stp.tile([1, 128], F32, tag="vv")
                nc.vector.tensor_tensor(out=vv, in0=t2r, in1=mu2,
                                        op=AL.subtract)
                lnv = stp.tile([1, 128], F32, tag="lnv")
                nc.scalar.activation(lnv, vv, AF.Ln)
                rrow = stp.tile([1, 128], F32, tag="rrow")
                nc.scalar.activation(rrow, lnv, AF.Exp, scale=-0.5)
                nc.vector.tensor_copy(
                    out=murow_bf[:, idx * 128:(idx + 1) * 128], in_=murow)
                rc = psT.tile([128, 1], F32, tag="rc")
                nc.tensor.transpose(rc, rrow, sb_of)
                nc.vector.tensor_copy(out=r_all[:, idx:idx + 1], in_=rc)

        psT_cm.__exit__(None, None, None)
        psC_cm.__exit__(None, None, None)
        psS_cm.__exit__(None, None, None)

        # ---------------- Phase C ----------------
        pc = ctx.enter_context(tc.tile_pool(name="pc", bufs=2))
        psP_cm = tc.tile_pool(name="psP", bufs=2, space="PSUM")
        psP = psP_cm.__enter__()
        for t, bri in (("c", 0), ("m", 1)):
            for nch in range(CD // 512):
                wg = pc.tile([128, H // 2, 512], BF16, tag="wg")
                nc.sync.dma_start(
                    wg,
                    Wg_d[t][:, nch * 512:(nch + 1) * 512]
                    .rearrange("(hp p) n -> p hp n", p=128))
                for bt in range(nb):
                    idx = bt * 2 + bri
                    px = psP.tile([128, 512], F32, tag="pxC")
                    for hp in range(H // 2):
                        nc.tensor.matmul(px,
                                         lhsT=caT2[:, idx * 8 + hp, :],
                                         rhs=wg[:, hp, :],
                                         start=(hp == 0), stop=False)
                    nc.tensor.matmul(
                        px,
                        lhsT=murow_bf[:, idx * 128:(idx + 1) * 128],
                        rhs=sb_un[t][:, nch * 512:(nch + 1) * 512],
                        start=False, stop=True)
                    t1 = pc.tile([128, 512], F32, tag="t1")
                    nc.scalar.activation(t1, px, AF.Copy,
                                         scale=r_all[:, idx:idx + 1])
                    xs = pc.tile([128, 512], BF16, tag="xs")
                    nc.sync.dma_start(
                        xs, xin_d[t][bt * 128:(bt + 1) * 128,
                                     nch * 512:(nch + 1) * 512])
                    ot = pc.tile([128, 512], BF16, tag="ot")
                    nc.vector.tensor_tensor(out=ot, in0=t1, in1=xs, op=AL.add)
                    if with_bias:
                        ot2 = pc.tile([128, 512], BF16, tag="ot2")
                        nc.vector.tensor_tensor(
                            out=ot2, in0=ot,
                            in1=_bc(sb_vb[t][:, nch * 512:(nch + 1) * 512],
                                    128),
                            op=AL.add)
                        ot = ot2
                    nc.sync.dma_start(
                        out_d[t][bt * 128:(bt + 1) * 128,
                                 nch * 512:(nch + 1) * 512], ot)
        psP_cm.__exit__(None, None, None)
        if dbg:
            nc.sync.dma_start(
                dbg_T1[:, :].rearrange("p (jb b8 h) -> p jb b8 h",
                                       b8=8, h=H), T1)
            nc.sync.dma_start(
                dbg_T2[:, :].rearrange("p (jb b8 h) -> p jb b8 h",
                                       b8=8, h=H), T2)
            nc.sync.dma_start(
                dbg_caT2[:, :].rearrange("p (i b) -> p i b",
                                         i=2 * nb * (H // 2)), caT2)
            nc.sync.dma_start(dbg_r[:, :], r_all)
            nc.sync.dma_start(dbg_mu[:, :], murow_bf)
            nc.sync.dma_start(dbg_v[:, :], v_dram["m"][:, :])
    return nc


_NC = {}


def _get_nc(with_bias):
    key = bool(with_bias)
    if key not in _NC:
        nc = build_nc(with_bias=key)
        if not nc.is_finalized():
            nc.finalize()
        _NC[key] = nc
    return _NC[key]


def _wb(consts):
    if "bq_c" not in consts:
        return False
    return bool(np.abs(consts["bq_c"]).max() > 0
                or np.abs(consts["bq_m"]).max() > 0
                or np.abs(consts["vb_c"]).max() > 0
                or np.abs(consts["vb_m"]).max() > 0)


def _host_prep(inputs):
    f32 = np.float32
    bf = ml_dtypes.bfloat16
    g = {k: np.asarray(v) for k, v in inputs.items()}
    consts = {}
    # Wg row permutation: device-hid p = hp*128 + par*64 + d
    #                     <-> reference hid r = d*16 + (2*hp + par)
    pdev = np.arange(HID)
    hp_, rem = pdev // 128, pdev % 128
    par_, d_ = rem // 64, rem % 64
    pr = d_ * H + 2 * hp_ + par_
    for t, (Wp, bp, g1, be1) in (
            ("c", ("W_cproj", "b_cproj", "g1", "be1")),
            ("m", ("W_mproj", "b_mproj", "g2", "be2"))):
        W = np.asarray(g[Wp], f32)
        g1d = np.asarray(g[g1], f32)
        be1d = np.asarray(g[be1], f32)
        gW = g1d[:, None] * W
        consts[f"Wg_{t}"] = np.ascontiguousarray(gW[pr, :]).astype(bf)
        consts[f"un_{t}"] = (-gW.sum(0)).reshape(1, CD).astype(bf)
        consts[f"vb_{t}"] = (be1d @ W + np.asarray(g[bp], f32)).reshape(1, CD)\
            .astype(f32)
    consts["Wq_c"] = np.asarray(g["W_cqkv"], f32).astype(bf)
    consts["Wq_m"] = np.asarray(g["W_mqkv"], f32).astype(bf)
    consts["bq_c"] = np.asarray(g["b_cqkv"], f32).reshape(1, 3 * HID)
    consts["bq_m"] = np.asarray(g["b_mqkv"], f32).reshape(1, 3 * HID)
    p = np.arange(128)
    consts["mask01"] = (p[:, None] // 16 == p[None, :] // 16)\
        .astype(f32).astype(bf)
    consts["ones16"] = np.ones((128, 16), bf)
    consts["ones_col"] = np.ones((128, 1), bf)
    consts["identb"] = np.eye(128).astype(bf)
    consts["ones_f"] = np.ones((1, 1), f32)
    return g, consts


def kernel(**inputs):
    g, consts = _host_prep(inputs)
    bf = ml_dtypes.bfloat16
    xc = np.ascontiguousarray(np.asarray(g["cnn_out"], np.float32)).astype(bf)
    xm = np.ascontiguousarray(np.asarray(g["mlp_out"], np.float32)).astype(bf)
    wb = _wb(consts)
    nc = _get_nc(wb)
    if not wb:
        consts = {k: v for k, v in consts.items()
                  if k not in ("bq_c", "bq_m", "vb_c", "vb_m")}
    in_maps = []
    for i in range(NCORES):
        m = dict(consts)
        m["x_c"] = xc[i * BS:(i + 1) * BS]
        m["x_m"] = xm[i * BS:(i + 1) * BS]
        in_maps.append(m)
    res = run_bass_kernel_spmd(nc, in_maps, list(range(NCORES))).results
    out_c = np.concatenate(
        [np.asarray(res[i]["out_c"]) for i in range(NCORES)], 0)
    out_m = np.concatenate(
        [np.asarray(res[i]["out_m"]) for i in range(NCORES)], 0)
    return (out_c.astype(np.float32), out_m.astype(np.float32))


# revision 23
# speedup vs baseline: 1.9631x; 1.0073x over previous
"""MultiHeadCrossAttentionFusion kernel for TRN2 (8 NeuronCores, data-parallel).

v3 design (per core, batch shard BS=1024):
  Phase A: xT via DMA-transpose (bf16 x from host). Q,K computed d-major
           directly on the PE (lhsT=W chunk, rhs=xT) into packed SBUF layout
           [part(d|branch), jblock, head, b8] so per-j score operands are
           contiguous 128-col slices. V computed in natural layout -> DRAM.
  B+C interleaved per (branch, 128-row tile): VP [(g,b8), j, d|1] gathered by
           DMA from v DRAM with ones column. Per j: score matmul, exp on ACT,
           block-diag mask on GpSimd, ca+denominator in one matmul
           (rhs=VP[:,j,0:65]), per-partition softmax scale, PE transpose,
           parity-split (vector+scalar) into a per-tile caT2. LN stats via
           ones-matmuls into one PSUM bank; rsqrt = exp(-0.5*ln(v)) with the
           combined natural_log_exp table set forced (no table thrash).
           Projection immediately follows per tile (Wg resident in SBUF):
           8 caT2 matmuls + mu*un, r_col scale on ACT, residual add on GpSimd,
           bf16 out. Host casts outputs back to f32.
"""
import sys
sys.path.insert(0, "/opt/trn_rl_repo")
import functools
import numpy as np
import ml_dtypes
from contextlib import ExitStack

import concourse.bass as bass
from concourse import bacc as _bacc
import concourse.hw_specs as _hw_specs
import concourse.mybir as mybir
from concourse.tile import TileContext
from concourse.bass_utils import run_bass_kernel_spmd

# Force Exp and Ln to resolve to the combined table set so the ACT engine
# never reloads tables mid-kernel.
_ORIG_GAT = _hw_specs.get_activation_tables


@functools.cache
def _gat_combined(arch):
    t = dict(_ORIG_GAT(arch))
    if "natural_log_exp_and_others" in t:
        for name in ("exp_and_others", "exp_and_friends", "natural_log"):
            if name in t:
                t[name] = set()
    return t


_hw_specs.get_activation_tables = _gat_combined
_bacc.get_activation_tables = _gat_combined

B, CD, HID, H, D = 8192, 2048, 1024, 16, 64
NCORES = 8
BS = B // NCORES          # 1024 rows per core
KT = CD // 128            # 16 k-tiles
EPS = 1e-5
F32 = mybir.dt.float32
BF16 = mybir.dt.bfloat16
AL = mybir.AluOpType
AF = mybir.ActivationFunctionType


def _bc(ap, p):
    """Broadcast a [1, ...] AP across p partitions."""
    return bass.AP(tensor=ap.tensor, offset=ap.offset,
                   ap=[[0, p]] + list(ap.ap)[1:])


def build_nc(with_bias=False, bs=BS, dbg=False):
    nb = bs // 128
    bchunk = min(512, bs)
    nbc = bs // bchunk
    nc = _bacc.Bacc()
    dp = nc.declare_dram_parameter
    x_c = dp("x_c", [bs, CD], BF16, isOutput=False)
    x_m = dp("x_m", [bs, CD], BF16, isOutput=False)
    Wq_c = dp("Wq_c", [CD, 3 * HID], BF16, isOutput=False)
    Wq_m = dp("Wq_m", [CD, 3 * HID], BF16, isOutput=False)
    Wg_c = dp("Wg_c", [HID, CD], BF16, isOutput=False)
    Wg_m = dp("Wg_m", [HID, CD], BF16, isOutput=False)
    un_c = dp("un_c", [1, CD], BF16, isOutput=False)
    un_m = dp("un_m", [1, CD], BF16, isOutput=False)
    mask01 = dp("mask01", [128, 128], BF16, isOutput=False)
    ones16 = dp("ones16", [128, 16], BF16, isOutput=False)
    ones_col = dp("ones_col", [128, 1], BF16, isOutput=False)
    identb = dp("identb", [128, 128], BF16, isOutput=False)
    ones_f = dp("ones_f", [1, 1], F32, isOutput=False)
    out_c = dp("out_c", [bs, CD], BF16, isOutput=True)
    out_m = dp("out_m", [bs, CD], BF16, isOutput=True)
    if dbg:
        dbg_T1 = dp("dbg_T1", [128, H * bs], BF16, isOutput=True)
        dbg_T2 = dp("dbg_T2", [128, H * bs], BF16, isOutput=True)
        dbg_v = dp("dbg_v", [bs, HID], BF16, isOutput=True)
    if with_bias:
        bq_c = dp("bq_c", [1, 3 * HID], F32, isOutput=False)
        bq_m = dp("bq_m", [1, 3 * HID], F32, isOutput=False)
        vb_c = dp("vb_c", [1, CD], F32, isOutput=False)
        vb_m = dp("vb_m", [1, CD], F32, isOutput=False)

    xin_d = {"c": x_c, "m": x_m}
    Wq_d = {"c": Wq_c, "m": Wq_m}
    Wg_d = {"c": Wg_c, "m": Wg_m}
    un_d = {"c": un_c, "m": un_m}
    out_d = {"c": out_c, "m": out_m}
    # partition bases: T1 holds q_c @0, q_m @64; T2 holds k_m @0, k_c @64
    qbase = {"c": 0, "m": 64}
    kbase = {"m": 0, "c": 64}

    with TileContext(nc) as tc, ExitStack() as ctx:
        consts = ctx.enter_context(tc.tile_pool(name="consts", bufs=1))
        dram = ctx.enter_context(tc.tile_pool(name="dram", bufs=1, space="DRAM"))
        keep = ctx.enter_context(tc.tile_pool(name="keep", bufs=1))

        sb_mask = consts.tile([128, 128], BF16)
        nc.sync.dma_start(sb_mask, mask01[:, :])
        sb_id = consts.tile([128, 128], BF16)
        nc.sync.dma_start(sb_id, identb[:, :])
        sb_o16 = consts.tile([128, 16], BF16)
        nc.sync.dma_start(sb_o16, ones16[:, :])
        sb_ones = consts.tile([128, 1], BF16)
        nc.sync.dma_start(sb_ones, ones_col[:, :])
        sb_of = consts.tile([1, 1], F32)
        nc.sync.dma_start(sb_of, ones_f[:, :])
        sb_un = {}
        for t in ("c", "m"):
            sb_un[t] = consts.tile([1, CD], BF16, name=f"un_{t}", tag=f"un_{t}")
            nc.sync.dma_start(sb_un[t], un_d[t][:, :])
        sb_bqc = {}
        sb_bqr = {}
        sb_vb = {}
        if with_bias:
            for t, bq, vb in (("c", bq_c, vb_c), ("m", bq_m, vb_m)):
                sb_bqc[t] = consts.tile([128, 2 * HID // 128], F32,
                                        name=f"bqc_{t}", tag=f"bqc_{t}")
                nc.sync.dma_start(
                    sb_bqc[t],
                    bq[:, 0:2 * HID].rearrange("o (c p) -> (o p) c", p=128))
                sb_bqr[t] = consts.tile([1, 3 * HID], F32,
                                        name=f"bqr_{t}", tag=f"bqr_{t}")
                nc.sync.dma_start(sb_bqr[t], bq[:, :])
                sb_vb[t] = consts.tile([1, CD], F32,
                                       name=f"vb_{t}", tag=f"vb_{t}")
                nc.sync.dma_start(sb_vb[t], vb[:, :])

        v_dram = {
            t: dram.tile([bs, HID], BF16, name=f"vd_{t}", tag=f"vd_{t}")
            for t in ("c", "m")
        }

        # packed layout [part, jblock, b8, head]: per-j matmul operands are
        # contiguous 128-column slices (col = b8*16 + head)
        T1 = keep.tile([128, bs // 8, 8, H], BF16, tag="T1")   # QP
        T2 = keep.tile([128, bs // 8, 8, H], BF16, tag="T2")   # KP

        # Wg resident in SBUF for the interleaved projection
        wg_all = {}
        for t in ("c", "m"):
            wg_all[t] = keep.tile([128, H // 2, CD], BF16,
                                  name=f"wga_{t}", tag=f"wga_{t}")
            nc.sync.dma_start(
                wg_all[t],
                Wg_d[t][:, :].rearrange("(hp p) n -> p hp n", p=128))

        # ---------------- Phase A ----------------
        paX_cm = tc.tile_pool(name="paX", bufs=1)
        paX = paX_cm.__enter__()
        paW_cm = tc.tile_pool(name="paW", bufs=2)
        paW = paW_cm.__enter__()
        paT_cm = tc.tile_pool(name="paT", bufs=2)
        paT = paT_cm.__enter__()
        psA_cm = tc.tile_pool(name="psA", bufs=2, space="PSUM")
        psA = psA_cm.__enter__()

        for t in ("m", "c"):
            xT = paX.tile([128, KT, bs], BF16, tag="xT")
            for kt in range(KT):
                nc.sync.dma_start_transpose(
                    xT[:, kt, :], xin_d[t][:, kt * 128:(kt + 1) * 128])
            # q, k in d-major packed layout
            for sect, dtile, dbase in (("q", T1, qbase[t]),
                                       ("k", T2, kbase[t])):
                off = 0 if sect == "q" else HID
                for ch in range(2):
                    wst = paW.tile([128, KT, 512], BF16, tag="wst")
                    nc.sync.dma_start(
                        wst,
                        Wq_d[t][:, off + ch * 512:off + (ch + 1) * 512]
                        .rearrange("(kt p) n -> p kt n", p=128))
                    for hs in range(4):
                        hp = ch * 4 + hs
                        cc = off // 128 + ch * 4 + hs
                        for bc in range(nbc):
                            px = psA.tile([128, bchunk], F32, tag="pxA")
                            bsl = slice(bc * bchunk, (bc + 1) * bchunk)
                            for kt in range(KT):
                                nc.tensor.matmul(
                                    px,
                                    lhsT=wst[:, kt, hs * 128:(hs + 1) * 128],
                                    rhs=xT[:, kt, bsl],
                                    start=(kt == 0), stop=(kt == KT - 1))
                            jbc = bchunk // 8
                            for par in range(2):
                                src = px[par * 64:(par + 1) * 64, :]\
                                    .rearrange("p (jb b8) -> p jb b8", b8=8)
                                dst = dtile[dbase:dbase + 64,
                                            bc * jbc:(bc + 1) * jbc,
                                            :, 2 * hp + par]
                                if with_bias:
                                    nc.vector.tensor_scalar(
                                        out=dst, in0=src,
                                        scalar1=sb_bqc[t][
                                            par * 64:(par + 1) * 64,
                                            cc:cc + 1],
                                        scalar2=None, op0=AL.add)
                                elif par == 0:
                                    nc.vector.tensor_copy(out=dst, in_=src)
                                else:
                                    nc.scalar.copy(out=dst, in_=src)
            # v in natural layout -> DRAM
            for ch in range(2):
                wst = paW.tile([128, KT, 512], BF16, tag="wst")
                nc.sync.dma_start(
                    wst,
                    Wq_d[t][:, 2 * HID + ch * 512:2 * HID + (ch + 1) * 512]
                    .rearrange("(kt p) n -> p kt n", p=128))
                for bt in range(nb):
                    px = psA.tile([128, 512], F32, tag="pxV")
                    for kt in range(KT):
                        nc.tensor.matmul(
                            px,
                            lhsT=xT[:, kt, bt * 128:(bt + 1) * 128],
                            rhs=wst[:, kt, :],
                            start=(kt == 0), stop=(kt == KT - 1))
                    vs = paT.tile([128, 512], BF16, tag="vstage")
                    if with_bias:
                        nc.vector.tensor_tensor(
                            out=vs, in0=px,
                            in1=_bc(sb_bqr[t][:, 2 * HID + ch * 512:
                                              2 * HID + (ch + 1) * 512], 128),
                            op=AL.add)
                    else:
                        nc.vector.tensor_copy(out=vs, in_=px)
                    nc.sync.dma_start(
                        v_dram[t][bt * 128:(bt + 1) * 128,
                                  ch * 512:(ch + 1) * 512], vs)

        psA_cm.__exit__(None, None, None)
        paT_cm.__exit__(None, None, None)
        paW_cm.__exit__(None, None, None)
        paX_cm.__exit__(None, None, None)

        # ---------------- Phases B + C interleaved ----------------
        pb = ctx.enter_context(tc.tile_pool(name="pb", bufs=2))
        spool = ctx.enter_context(tc.tile_pool(name="spool", bufs=3))
        stp = ctx.enter_context(tc.tile_pool(name="stp", bufs=4))
        pcat = ctx.enter_context(tc.tile_pool(name="pcat", bufs=2))
        pc = ctx.enter_context(tc.tile_pool(name="pc", bufs=2))
        psS = ctx.enter_context(tc.tile_pool(name="psS", bufs=2, space="PSUM"))
        psC = ctx.enter_context(tc.tile_pool(name="psC", bufs=1, space="PSUM"))
        psT = ctx.enter_context(tc.tile_pool(name="psT", bufs=2, space="PSUM"))
        psK = ctx.enter_context(tc.tile_pool(name="psK", bufs=1, space="PSUM"))
        psP = ctx.enter_context(tc.tile_pool(name="psP", bufs=2, space="PSUM"))

        for bri, (qs, ks) in enumerate((("c", "m"), ("m", "c"))):
            t = "c" if bri == 0 else "m"
            qb = qbase[qs]
            kb = kbase[ks]
            for bt in range(nb):
                vp = pb.tile([128, 16, 65], BF16, tag="vp")
                nc.sync.dma_start(
                    vp[:, :, 0:64],
                    v_dram[ks][bt * 128:(bt + 1) * 128, :].rearrange(
                        "(j b8) (g d) -> (b8 g) j d", b8=8, g=16))
                nc.vector.tensor_copy(out=vp[:, :, 64:65], in_=sb_o16)
                caT2 = pcat.tile([128, H // 2, 128], BF16, tag="caT2")

                for j in range(16):
                    jb = bt * 16 + j
                    sp = psS.tile([128, 128], F32, tag="sp")
                    nc.tensor.matmul(
                        sp,
                        lhsT=T2[kb:kb + 64, jb, :, :],
                        rhs=T1[qb:qb + 64, jb, :, :],
                        start=True, stop=True)
                    eT = spool.tile([128, 128], BF16, tag="eT")
                    nc.scalar.activation(eT, sp, AF.Exp, scale=0.125)
                    eTm = spool.tile([128, 128], BF16, tag="eTm")
                    nc.gpsimd.tensor_tensor(out=eTm, in0=eT, in1=sb_mask,
                                            op=AL.mult)
                    cu = psC.tile([128, 65], F32, tag="cu")
                    nc.tensor.matmul(cu, lhsT=eTm, rhs=vp[:, j, :],
                                     start=True, stop=True)
                    rcz = stp.tile([128, 1], F32, tag="rcz")
                    nc.vector.reciprocal(rcz, cu[:, 64:65])
                    caj = spool.tile([128, 64], BF16, tag="caj")
                    nc.vector.tensor_scalar(out=caj, in0=cu[:, 0:64],
                                            scalar1=rcz, scalar2=None,
                                            op0=AL.mult)
                    ct = psT.tile([64, 128], BF16, tag="ct")
                    nc.tensor.transpose(ct, caj, sb_id)
                    # cols of ct: b8*16 + 2*hp + par
                    for par in range(2):
                        src = ct.rearrange(
                            "d (b8 hp two) -> d hp b8 two",
                            b8=8, two=2)[:, :, :, par]
                        dst = caT2[par * 64:(par + 1) * 64, :,
                                   j * 8:(j + 1) * 8]
                        if par == 0:
                            nc.vector.tensor_copy(out=dst, in_=src)
                        else:
                            nc.scalar.copy(out=dst, in_=src)

                # LN stats: one PSUM bank holds rc col + mrow + srow
                sq = spool.tile([128, H // 2, 128], BF16, tag="sq")
                nc.gpsimd.tensor_tensor(out=sq, in0=caT2, in1=caT2,
                                        op=AL.mult)
                kitchen = psK.tile([128, 512], F32, tag="kitchen")
                mrow = kitchen[0:1, 128:256]
                srow = kitchen[0:1, 256:384]
                for hp in range(H // 2):
                    nc.tensor.matmul(mrow, lhsT=sb_ones,
                                     rhs=caT2[:, hp, :],
                                     start=(hp == 0), stop=(hp == 7))
                for hp in range(H // 2):
                    nc.tensor.matmul(srow, lhsT=sb_ones, rhs=sq[:, hp, :],
                                     start=(hp == 0), stop=(hp == 7))
                murow = stp.tile([1, 128], F32, tag="murow")
                nc.vector.tensor_scalar(out=murow, in0=mrow,
                                        scalar1=1.0 / HID, scalar2=None,
                                        op0=AL.mult)
                t2r = stp.tile([1, 128], F32, tag="t2r")
                nc.vector.tensor_scalar(out=t2r, in0=srow, scalar1=1.0 / HID,
                                        scalar2=EPS, op0=AL.mult, op1=AL.add)
                mu2 = stp.tile([1, 128], F32, tag="mu2")
                nc.vector.tensor_tensor(out=mu2, in0=murow, in1=murow,
                                        op=AL.mult)
                vv = stp.tile([1, 128], F32, tag="vv")
                nc.vector.tensor_tensor(out=vv, in0=t2r, in1=mu2,
                                        op=AL.subtract)
                lnv = stp.tile([1, 128], F32, tag="lnv")
                nc.scalar.activation(lnv, vv, AF.Ln)
                rrow = stp.tile([1, 128], F32, tag="rrow")
                nc.scalar.activation(rrow, lnv, AF.Exp, scale=-0.5)
                mu_bf = stp.tile([1, 128], BF16, tag="mu_bf")
                nc.vector.tensor_copy(out=mu_bf, in_=murow)
                rc = kitchen[:, 0:1]
                nc.tensor.transpose(rc, rrow, sb_of)
                r_col = stp.tile([128, 1], F32, tag="r_col")
                nc.vector.tensor_copy(out=r_col, in_=rc)

                # ---- projection for this tile (C) ----
                for nch in range(CD // 512):
                    nsl = slice(nch * 512, (nch + 1) * 512)
                    px = psP.tile([128, 512], F32, tag="pxC")
                    for hp in range(H // 2):
                        nc.tensor.matmul(px,
                                         lhsT=caT2[:, hp, :],
                                         rhs=wg_all[t][:, hp, nsl],
                                         start=(hp == 0), stop=False)
                    nc.tensor.matmul(
                        px, lhsT=mu_bf, rhs=sb_un[t][:, nsl],
                        start=False, stop=True)
                    t1 = pc.tile([128, 512], F32, tag="t1")
                    nc.scalar.activation(t1, px, AF.Copy, scale=r_col)
                    xs = pc.tile([128, 512], BF16, tag="xs")
                    nc.sync.dma_start(
                        xs, xin_d[t][bt * 128:(bt + 1) * 128, nsl])
                    ot = pc.tile([128, 512], BF16, tag="ot")
                    nc.gpsimd.tensor_tensor(out=ot, in0=t1, in1=xs, op=AL.add)
                    if with_bias:
                        ot2 = pc.tile([128, 512], BF16, tag="ot2")
                        nc.vector.tensor_tensor(
                            out=ot2, in0=ot,
                            in1=_bc(sb_vb[t][:, nsl], 128),
                            op=AL.add)
                        ot = ot2
                    nc.sync.dma_start(
                        out_d[t][bt * 128:(bt + 1) * 128, nsl], ot)

        if dbg:
            nc.sync.dma_start(
                dbg_T1[:, :].rearrange("p (jb b8 h) -> p jb b8 h",
                                       b8=8, h=H), T1)
            nc.sync.dma_start(
                dbg_T2[:, :].rearrange("p (jb b8 h) -> p jb b8 h",
                                       b8=8, h=H), T2)
            nc.sync.dma_start(dbg_v[:, :], v_dram["m"][:, :])
    return nc


_NC = {}


def _get_nc(with_bias):
    key = bool(with_bias)
    if key not in _NC:
        nc = build_nc(with_bias=key)
        if not nc.is_finalized():
            nc.finalize()
        _NC[key] = nc
    return _NC[key]


def _wb(consts):
    if "bq_c" not in consts:
        return False
    return bool(np.abs(consts["bq_c"]).max() > 0
                or np.abs(consts["bq_m"]).max() > 0
                or np.abs(consts["vb_c"]).max() > 0
                or np.abs(consts["vb_m"]).max() > 0)


def _host_prep(inputs):
    f32 = np.float32
    bf = ml_dtypes.bfloat16
    g = {k: np.asarray(v) for k, v in inputs.items()}
    consts = {}
    # Wg row permutation: device-hid p = hp*128 + par*64 + d
    #                     <-> reference hid r = d*16 + (2*hp + par)
    pdev = np.arange(HID)
    hp_, rem = pdev // 128, pdev % 128
    par_, d_ = rem // 64, rem % 64
    pr = d_ * H + 2 * hp_ + par_
    for t, (Wp, bp, g1, be1) in (
            ("c", ("W_cproj", "b_cproj", "g1", "be1")),
            ("m", ("W_mproj", "b_mproj", "g2", "be2"))):
        W = np.asarray(g[Wp], f32)
        g1d = np.asarray(g[g1], f32)
        be1d = np.asarray(g[be1], f32)
        gW = g1d[:, None] * W
        consts[f"Wg_{t}"] = np.ascontiguousarray(gW[pr, :]).astype(bf)
        consts[f"un_{t}"] = (-gW.sum(0)).reshape(1, CD).astype(bf)
        consts[f"vb_{t}"] = (be1d @ W + np.asarray(g[bp], f32)).reshape(1, CD)\
            .astype(f32)
    consts["Wq_c"] = np.asarray(g["W_cqkv"], f32).astype(bf)
    consts["Wq_m"] = np.asarray(g["W_mqkv"], f32).astype(bf)
    consts["bq_c"] = np.asarray(g["b_cqkv"], f32).reshape(1, 3 * HID)
    consts["bq_m"] = np.asarray(g["b_mqkv"], f32).reshape(1, 3 * HID)
    p = np.arange(128)
    consts["mask01"] = (p[:, None] // 16 == p[None, :] // 16)\
        .astype(f32).astype(bf)
    consts["ones16"] = np.ones((128, 16), bf)
    consts["ones_col"] = np.ones((128, 1), bf)
    consts["identb"] = np.eye(128).astype(bf)
    consts["ones_f"] = np.ones((1, 1), f32)
    return g, consts


def kernel(**inputs):
    g, consts = _host_prep(inputs)
    bf = ml_dtypes.bfloat16
    xc = np.ascontiguousarray(np.asarray(g["cnn_out"], np.float32)).astype(bf)
    xm = np.ascontiguousarray(np.asarray(g["mlp_out"], np.float32)).astype(bf)
    wb = _wb(consts)
    nc = _get_nc(wb)
    if not wb:
        consts = {k: v for k, v in consts.items()
                  if k not in ("bq_c", "bq_m", "vb_c", "vb_m")}
    in_maps = []
    for i in range(NCORES):
        m = dict(consts)
        m["x_c"] = xc[i * BS:(i + 1) * BS]
        m["x_m"] = xm[i * BS:(i + 1) * BS]
        in_maps.append(m)
    res = run_bass_kernel_spmd(nc, in_maps, list(range(NCORES))).results
    out_c = np.concatenate(
        [np.asarray(res[i]["out_c"]) for i in range(NCORES)], 0)
    out_m = np.concatenate(
        [np.asarray(res[i]["out_m"]) for i in range(NCORES)], 0)
    return (out_c.astype(np.float32), out_m.astype(np.float32))
